# revision 2
# baseline (speedup 1.0000x reference)
"""Trainium2 Bass kernel for nn_Diagomal_DWConv (diagonal depthwise conv).

Math (derived from the reference):
  View x as rows X[r, w], r in [0, R), R = B*C*H, W columns.
  out[r, w] = bias[c(r)] + sum_i weight[c(r), 0, i] * X[(r + 2 - i) mod R, w + i - 2]
  with zero padding in w only, c(r) = (r // H) mod C.

Strategy (banded-stationary matmul, all transfers bf16):
  - One batch (16384 rows) per NeuronCore, processed in windows of 128
    consecutive rows (124 output rows per window, halo taps included).
  - The host pre-skews each window group: SBUF partition q holds padded row
    row0+q at free offset q, so the 5-tap diagonal conv becomes ONE bf16
    matmul per window with a 5-banded stationary S[q, p] = w[c, p+4-q]
    (vs 5 diag fp32r matmuls per 2 rows in the old kernel -> ~5x less PE).
    Input DMAs are plain 2D copies of the pre-skewed image: 128 fat
    descriptors per group (HWDGE descriptor generation is ~4ns/desc and
    serializes per queue, so descriptor count is the scarce resource).
  - The band depends only on q-p and the channel weights, so all windows
    share 64 per-channel stationary images loaded once at start. A window
    whose 124 output rows straddle a channel boundary issues two matmuls
    with partition-sliced PSUM outputs.
  - PSUM tiles span 4 banks (4 windows); PSUM->SBUF bf16 drains handle 4
    windows per op on alternating ACT/DVE engines to amortize op overhead.
  - Output rows sit skewed in staging (row p of window j at
    ot[p, j*384 + p + w + 2]); the output DMA un-skews with a
    partition-stride pitch+1 *read* (legal on HW; skewed *writes* wrap
    their per-partition byte offset mod 8 and are broken) and rides the
    gpsimd SWDGE ring (0.34ns/desc) like the old kernel's output did.
  - Everything crosses HBM in bf16 (tolerance 2e-2, bf16 keeps us ~4e-3);
    the per-channel bias is added on the host after the gather.
"""

import numpy as np
import ml_dtypes

import concourse.bacc as bacc
import concourse.tile as tile
import concourse.mybir as mybir
from concourse.bass_utils import run_bass_kernel_spmd

F32 = mybir.dt.float32
BF16 = mybir.dt.bfloat16
BF16NP = ml_dtypes.bfloat16

B, C, H, W = 8, 64, 256, 256
KS, PAD = 5, 2
R = B * C * H            # 131072 rows total
NCORES = 8
RC = R // NCORES         # 16384 rows per core (exactly one batch)
NP_ = 128                # partitions
WROWS = NP_ - KS + 1     # 124 valid output rows per 128-row window
NW = -(-RC // WROWS)     # 133 windows per core
GW = 12                  # windows per group
NG = -(-NW // GW)        # 12 groups (11 full + 1 single-window)
PR = 260                 # padded row length (2 + 256 + 2)
IW = GW * PR + 136       # input image width per partition (3256)
SPITCH = 132             # per-channel stationary pitch
OP = 384                 # per-window psum/staging width (f = p + w + 2)
PBANK = 512              # psum bank width in f32
QW = 2                   # windows per psum tile / drain op
NWARM = 64               # PE warm-up matmuls

_CACHE = {}


def _build_nc():
    nc = bacc.Bacc("TRN2", num_devices=NCORES)
    xk = nc.dram_tensor("xk", [(11 * IW + (PR + 136)) * NP_], BF16, kind="ExternalInput")
    sk = nc.dram_tensor("sk", [NP_, NW * SPITCH], BF16, kind="ExternalInput")
    yk = nc.dram_tensor("yk", [NP_ * NW * OP], BF16, kind="ExternalOutput")

    with tile.TileContext(nc) as tc:
        with (
            tc.tile_pool(name="slab", bufs=6) as spool,
            tc.tile_pool(name="stat", bufs=6) as tpool,
            tc.tile_pool(name="outp", bufs=5) as opool,
            tc.tile_pool(name="ps", bufs=4, space="PSUM") as pspool,
            tc.tile_pool(name="warm", bufs=1) as wpool,
        ):
            # PE warm-up: dep-free tiny matmuls run during the DMA head so
            # the HAM clock gate is released before the real stream starts.
            wt_ = wpool.tile([NP_, 256], BF16)
            nc.vector.memset(wt_[:].bitcast(F32), 0.0)
            wps = pspool.tile([NP_, 256], F32, tag="ps")
            for _ in range(NWARM):
                nc.tensor.matmul(
                    wps[0:64, 0:256], wt_[:, 0:64], wt_[:, 0:256], start=True, stop=True
                )

            drains = [nc.scalar, nc.vector]
            dri = 0
            for g in range(NG):
                w0 = g * GW
                nwg = min(GW, NW - w0)

                # pre-skewed input image for this group (plain 2D copy)
                iw = nwg * PR + 136
                it = spool.tile([NP_, iw], BF16)
                isrc = xk.ap().copy()
                isrc.offset = g * IW * NP_
                isrc.ap = mybir.VecI64Pair([[iw, NP_], [1, iw]])
                nc.sync.dma_start(it[:], isrc)

                st = tpool.tile([NP_, nwg * SPITCH], BF16)
                ssrc = sk.ap().copy()
                ssrc.offset = w0 * SPITCH
                ssrc.ap = mybir.VecI64Pair([[NW * SPITCH, NP_], [1, nwg * SPITCH]])
                nc.sync.dma_start(st[:], ssrc)

                ot = opool.tile([NP_, nwg * OP], BF16)
                j = 0
                while j < nwg:
                    nq = min(QW, nwg - j)
                    ps = pspool.tile([NP_, nq * PBANK], F32)
                    for u in range(nq):
                        k = w0 + j + u
                        r0 = k * WROWS
                        mov = it[:, (j + u) * PR + 2 : (j + u) * PR + 2 + OP]
                        pv = ps[:, u * PBANK : u * PBANK + OP]
                        jl = j + u
                        nc.tensor.matmul(
                            pv,
                            st[:, jl * SPITCH + 4 : jl * SPITCH + 4 + NP_],
                            mov,
                            start=True,
                            stop=True,
                        )
                    # drain nq windows in one op (multi-bank strided read)
                    pin = ps[:].copy()
                    pin.ap = mybir.VecI64Pair([[nq * PBANK, NP_], [PBANK, nq], [1, OP]])
                    pout = ot[:].copy()
                    pout.ap = mybir.VecI64Pair([[nwg * OP, NP_], [OP, nq], [1, OP]])
                    pout.offset = j * OP
                    eng = drains[dri % 2]
                    dri += 1
                    if eng is nc.scalar:
                        eng.copy(pout, pin)
                    else:
                        eng.tensor_copy(pout, pin)
                    j += nq

                # output: ship the whole skewed staging block with fat
                # per-partition descriptors (desc processing, not bytes, is
                # the scarce DMA resource); the host un-skews row p of
                # window k from yk[p, k, p + 2 : p + 258].
                odst = yk.ap().copy()
                odst.offset = w0 * OP
                odst.ap = mybir.VecI64Pair([[NW * OP, NP_], [1, nwg * OP]])
                nc.gpsimd.dma_start(odst, ot[:])

    nc.compile()
    return nc


def _host_prep(x, weight, bias):
    """Per-core in_maps: pre-skewed padded row images + channel band image."""
    xr = np.ascontiguousarray(x, dtype=np.float32).reshape(R, W).astype(BF16NP)
    big = np.zeros((R + 4 + 124, PR), dtype=BF16NP)
    big[2 : R + 2, 2 : 2 + W] = xr
    big[0:2, 2 : 2 + W] = xr[R - 2 : R]
    big[R + 2 : R + 4, 2 : 2 + W] = xr[0:2]

    # per-window band image: S[q, k*132 + q + jj] = weight[c(124k+q-4+jj), jj]
    wgt = np.ascontiguousarray(weight, dtype=np.float32).reshape(C, KS).astype(BF16NP)
    sk = np.zeros((NP_, NW * SPITCH), dtype=BF16NP)
    qq = np.arange(NP_)[:, None, None]
    kk = np.arange(NW)[None, :, None]
    jj = np.arange(KS)[None, None, :]
    rel = np.clip(WROWS * kk + qq - 4 + jj, 0, RC - 1)
    ch = (rel // H) % C
    np.put_along_axis(
        sk.reshape(NP_, NW, SPITCH),
        (qq + jj).astype(np.int64),
        wgt[ch, jj],
        axis=2,
    )

    idx = np.arange(NW)[None, :] * WROWS + np.arange(NP_)[:, None]  # [q, k]
    in_maps = []
    for kc in range(NCORES):
        pall = big[kc * RC : kc * RC + RC + 128]
        rows = pall[idx]  # [q, k, PR]
        img = np.zeros((11, NP_, IW), dtype=BF16NP)
        img11 = np.zeros((NP_, PR + 136), dtype=BF16NP)
        for q in range(NP_):
            img[:, q, q : q + GW * PR] = rows[q, : 11 * GW].reshape(11, GW * PR)
            img11[q, q : q + PR] = rows[q, 11 * GW]
        in_maps.append(
            {"xk": np.concatenate([img.reshape(-1), img11.reshape(-1)]), "sk": sk}
        )
    return in_maps


def kernel(x, weight, bias):
    x = np.asarray(x)
    weight = np.asarray(weight)
    bias = np.asarray(bias)
    if "nc" not in _CACHE:
        _CACHE["nc"] = _build_nc()
    nc = _CACHE["nc"]
    in_maps = _host_prep(x, weight, bias)
    res = run_bass_kernel_spmd(nc, in_maps, list(range(NCORES)))
    out = np.empty((NCORES, RC, W), dtype=np.float32)
    tmp = np.empty((WROWS, NW, W), dtype=BF16NP)
    for kc in range(NCORES):
        ya = np.asarray(res.results[kc]["yk"]).reshape(NP_, NW, OP)
        for p in range(WROWS):
            tmp[p] = ya[p, :, p + 2 : p + 2 + W]
        out[kc] = (
            tmp.transpose(1, 0, 2).reshape(NW * WROWS, W)[:RC].astype(np.float32)
        )
    out = out.reshape(B, C, H, W)
    out += np.asarray(bias, dtype=np.float32)[None, :, None, None]
    return out
